# revision 25
# baseline (speedup 1.0000x reference)
"""Multi-head attention (b=8, n=1024, dim=1024, 16 heads x 64) on 8 TRN2 cores.

Sharding: data-parallel over batch (one batch element per core), SPMD NEFF.

Per-core dataflow (all matmuls f32r = full-speed PE with ~1e-4 matmul error):
  A) xT = PE-transpose(x) (f32r, batched 512-wide PSUM evacuations);
     v[t,f] = xT.T @ w_v written straight into a persistent SBUF tile in
     PV-stationary layout [keys-part, ko, h*65] with a ones column per head
     (no DRAM round-trip);
  B) qkT[f,t] = w_qk.T @ xT in head-pair order, software-pipelined with
     per-pair attention: S^T[j,i] = kT.T @ qT (K=64, two heads row-tiled
     concurrently); P^T = exp(scale*S^T) on ACT; [outT|l] = [v|1].T @ P^T
     (M=65, softmax denominator for free); normalize via reciprocal +
     K=33 ones-broadcast matmul; normalized output overwrites the dead q
     rows of qkT so cat == qkT.
  C) out = cat.T @ w_out + b_out; w_out is prefetched at phase-B start.

Scheduling: one PSUM pool (tags io/mm/s = 2+2+4 banks); phase-A SBUF pools
close before phase B so w_out + pipeline tiles reuse their space.
"""
import sys

sys.path.insert(0, "/opt/trn_rl_repo")

import numpy as np

import concourse.mybir as mybir
import concourse.tile as tile
from concourse import bacc
from concourse.bass_utils import run_bass_kernel_spmd
from concourse.masks import make_identity

FP32 = mybir.dt.float32
F32R = mybir.dt.float32r
AF = mybir.ActivationFunctionType
MUL = mybir.AluOpType.mult
ADD = mybir.AluOpType.add

N = 1024      # seq len
D = 1024      # model dim
H = 16        # heads
HD = 64       # head dim
SCALE = HD ** -0.5
NC_ = 8       # n cores = batch
KC = D // 128  # 8 contraction chunks


def build_attention_nc(reps=1):
    # reps>1 repeats the whole body in-NEFF -- used only by bench_hw.py to
    # measure per-exec time as a slope that cancels dispatch overhead.
    nc = bacc.Bacc("TRN2", target_bir_lowering=False, debug=False, num_devices=1)

    x = nc.dram_tensor("x", [N, D], FP32, kind="ExternalInput").ap()
    w_qkv = nc.dram_tensor("w_qkv", [D, 3 * D], FP32, kind="ExternalInput").ap()
    w_out = nc.dram_tensor("w_out", [D, D], FP32, kind="ExternalInput").ap()
    b_out = nc.dram_tensor("b_out", [1, D], FP32, kind="ExternalInput").ap()
    out = nc.dram_tensor("out", [N, D], FP32, kind="ExternalOutput").ap()

    with tile.TileContext(nc) as tc:
        for _rep in range(reps):
            _build_body(nc, tc, x, w_qkv, w_out, b_out, out)

    nc.compile()
    return nc


def _build_body(nc, tc, x, w_qkv, w_out, b_out, out):
        with (
            tc.tile_pool(name="persist", bufs=1) as pp,
            tc.tile_pool(name="ps", bufs=2, space="PSUM") as ps,
        ):
            ident = pp.tile([128, 128], FP32, tag="ident")
            make_identity(nc, ident[:])
            # f32r operands keep the broadcast matmuls at full PE speed
            # (fp32 matmuls run at 1/4 rate). memset can't write f32r, so
            # constants go through FP32 staging + converting copies.
            ones = pp.tile([1, 128], F32R, tag="ones")
            # head-pair selector for the paired 1/l broadcast: row 0 -> cols
            # 0-63 (even head), row 32 -> cols 64-127 (odd head)
            sel = pp.tile([33, 128], F32R, tag="sel")
            # reciprocal landing tiles: rows 0/32 are rewritten per use; rows
            # 1-31 must stay zero for the K=33 broadcast matmul
            rec_tiles = [
                pp.tile([33, 512], F32R, tag=f"rec{i}", name=f"rec{i}")
                for i in range(2)
            ]

            # xT[d % 128, kc, t]
            xT = pp.tile([128, KC, N], F32R, tag="xt", name="xT")
            qkT = [pp.tile([128, N], F32R, tag=f"qk{f}", name=f"qk{f}") for f in range(H)]
            # v in PV-stationary layout: [token%128, ko, h*65+e]; e==64 is 1.0
            v_sb = pp.tile([128, KC, H * 65], F32R, tag="vsb", name="v_sb")

            with tc.tile_pool(name="pi", bufs=1) as pi:
                sel_src = pi.tile([33, 128], FP32, tag="selsrc")
                nc.vector.memset(sel_src[:], 0.0)
                nc.vector.memset(sel_src[0:1, 0:64], 1.0)
                nc.vector.memset(sel_src[32:33, 64:128], 1.0)
                nc.vector.tensor_copy(sel[:], sel_src[:])
                zero_src = pi.tile([33, 512], FP32, tag="zerosrc")
                nc.vector.memset(zero_src[:], 0.0)
                for rt in rec_tiles:
                    nc.vector.tensor_copy(rt[:], zero_src[:])
                one_src = pi.tile([128, 128], FP32, tag="onesrc")
                nc.vector.memset(one_src[:], 1.0)
                nc.vector.tensor_copy(ones[:], one_src[0:1, :])
                nc.vector.tensor_copy(
                    v_sb[:].rearrange("p k (h e) -> p k h e", e=65)
                    [:, :, :, 64:65].rearrange("p k h e -> p k (h e)"),
                    one_src[:].rearrange("p (k h) -> p k h", k=KC),
                )

            # ---------------- Phase A: xT, v ----------------
            with tc.tile_pool(name="pa", bufs=1) as pa, \
                 tc.tile_pool(name="pa2", bufs=3) as pa2:
                # x chunk 0 first, then w_v's first half so v can start early
                x_tiles = [pa2.tile([128, D], FP32, tag="x", name="x0")]
                nc.sync.dma_start(x_tiles[0][:], x[0:128, :])
                wv = [pa.tile([128, KC, 512], F32R, tag=f"wv{fs}", name=f"wv{fs}") for fs in range(2)]
                nc.sync.dma_start(
                    wv[0][:],
                    w_qkv[:, 2 * D:2 * D + 512]
                    .rearrange("(ko p) f -> p ko f", p=128)
                    .bitcast(F32R),
                )
                for tc_i in range(1, 3):
                    x_sb = pa2.tile([128, D], FP32, tag="x", name=f"x{tc_i}")
                    nc.sync.dma_start(x_sb[:], x[tc_i * 128:(tc_i + 1) * 128, :])
                    x_tiles.append(x_sb)
                nc.sync.dma_start(
                    wv[1][:],
                    w_qkv[:, 2 * D + 512:2 * D + 1024]
                    .rearrange("(ko p) f -> p ko f", p=128)
                    .bitcast(F32R),
                )

                # warm the ACT exp table before phase B needs it
                warm = pa.tile([1, 2], FP32, tag="warm")
                nc.scalar.activation(warm[:], ident[0:1, 0:2], AF.Exp)

                for tc_i in range(8):
                    if tc_i >= 3:
                        x_sb = pa2.tile([128, D], FP32, tag="x", name=f"x{tc_i}")
                        nc.sync.dma_start(x_sb[:], x[tc_i * 128:(tc_i + 1) * 128, :])
                        x_tiles.append(x_sb)
                    x_sb = x_tiles[tc_i]
                    # transpose 4 d-chunks per PSUM tile, evacuate 512 wide
                    for half in range(2):
                        tr_ps = ps.tile([128, 512], FP32, tag="io", name="tr")
                        for q in range(4):
                            dc = half * 4 + q
                            nc.tensor.transpose(
                                tr_ps[:, q * 128:(q + 1) * 128],
                                x_sb[:, dc * 128:(dc + 1) * 128],
                                ident[:],
                            )
                        nc.vector.tensor_copy(
                            xT[:, half * 4:(half + 1) * 4, tc_i * 128:(tc_i + 1) * 128],
                            tr_ps[:].rearrange("p (a b) -> p a b", b=128),
                        )
                    for fs in range(2):
                        mm = ps.tile([128, 512], FP32, tag="mm")
                        for kc in range(KC):
                            nc.tensor.matmul(
                                mm[:],
                                xT[:, kc, tc_i * 128:(tc_i + 1) * 128],
                                wv[fs][:, kc, :],
                                start=(kc == 0),
                                stop=(kc == KC - 1),
                            )
                        nc.vector.tensor_copy(
                            v_sb[:, tc_i].rearrange("p (h e) -> p h e", e=65)
                            [:, fs * 8:(fs + 1) * 8, 0:64],
                            mm[:].rearrange("p (h e) -> p h e", e=64),
                        )

            # ---------------- Phase B/C pools ----------------
            with (
                tc.tile_pool(name="pwo", bufs=1) as pwo,
                tc.tile_pool(name="pbp", bufs=3) as pbp,
                tc.tile_pool(name="pb", bufs=2) as pb,
                tc.tile_pool(name="pb2", bufs=2) as pb2,
            ):
                # prefetch w_out + bias during phase B
                w_out_sb = pwo.tile([128, KC, D], F32R, tag="wo", name="w_out_sb")
                b_row = pwo.tile([1, D], F32R, tag="brow")
                b_sb = pwo.tile([128, D], FP32, tag="bsb")



                # Software pipeline: emit qk projection for head-pair f,
                # then the attention blocks for pair f-1. The exp stream on
                # ACT overlaps the qk matmuls on PE. Normalized attention
                # output is written back into the (dead) q rows of qkT, so
                # cat[c] IS qkT[c] (disjoint partition halves per head).
                def emit_qk(f):
                    for fc in (f, 8 + f):
                        wc = pb.tile([128, KC, 128], F32R, tag="wc")
                        nc.sync.dma_start(
                            wc[:],
                            w_qkv[:, fc * 128:(fc + 1) * 128]
                            .rearrange("(ko p) f -> p ko f", p=128)
                            .bitcast(F32R),
                        )
                        for ic in range(2):
                            mm = ps.tile([128, 512], FP32, tag="mm")
                            for kc in range(KC):
                                nc.tensor.matmul(
                                    mm[:],
                                    wc[:, kc, :],
                                    xT[:, kc, ic * 512:(ic + 1) * 512],
                                    start=(kc == 0),
                                    stop=(kc == KC - 1),
                                )
                            nc.vector.tensor_copy(qkT[fc][:, ic * 512:(ic + 1) * 512], mm[:])

                def emit_attention_pair(fp):
                    h_e, h_o = 2 * fp, 2 * fp + 1
                    qc = fp
                    qt, kt = qkT[qc], qkT[8 + qc]
                    for ic in range(2):
                        o_e = ps.tile([128, 512], FP32, tag="io", name="oe")
                        o_o = ps.tile([128, 512], FP32, tag="io", name="oo")
                        o_of = {h_e: o_e, h_o: o_o}
                        pts = []
                        # S matmuls for the two heads sit in disjoint PE row
                        # groups (partitions 0-63 / 64-127) -> HW-concurrent
                        for jc in range(KC):
                            s_ps = ps.tile([128, 2, 512], FP32, tag="s")
                            for hi, po in ((0, 0), (1, 64)):
                                nc.tensor.matmul(
                                    s_ps[:, hi, :],
                                    kt[po:po + 64, jc * 128:(jc + 1) * 128],
                                    qt[po:po + 64, ic * 512:(ic + 1) * 512],
                                    start=True,
                                    stop=True,
                                )
                            pt = pbp.tile([128, 2, 512], F32R, tag="pt")
                            nc.scalar.activation(
                                pt[:].rearrange("p a b -> p (a b)"),
                                s_ps[:].rearrange("p a b -> p (a b)"),
                                AF.Exp,
                                scale=SCALE,
                            )
                            pts.append(pt)
                            if jc >= 1:
                                for hi, h in ((0, h_e), (1, h_o)):
                                    nc.tensor.matmul(
                                        o_of[h][0:65, :],
                                        v_sb[:, jc - 1, h * 65:(h + 1) * 65],
                                        pts[jc - 1][:, hi, :],
                                        start=(jc - 1 == 0),
                                        stop=False,
                                    )
                        for hi, h in ((0, h_e), (1, h_o)):
                            nc.tensor.matmul(
                                o_of[h][0:65, :],
                                v_sb[:, KC - 1, h * 65:(h + 1) * 65],
                                pts[KC - 1][:, hi, :],
                                start=False,
                                stop=True,
                            )
                        rec2 = rec_tiles[ic]
                        # f32r out keeps the broadcast matmul at full PE rate;
                        # f32r is fp32-width so nothing is actually lost
                        with nc.allow_low_precision(reason="f32r == fp32 bits"):
                            nc.vector.reciprocal(rec2[0:1, :], o_e[64:65, :])
                            nc.vector.reciprocal(rec2[32:33, :], o_o[64:65, :])
                        b_ps = ps.tile([128, 512], FP32, tag="mm", name="bps")
                        nc.tensor.matmul(
                            b_ps[:], sel[:], rec2[:], start=True, stop=True
                        )
                        bc_sb = pb2.tile([128, 512], FP32, tag="bc")
                        nc.vector.tensor_copy(bc_sb[:], b_ps[:])
                        for hi, (h, po) in enumerate(((h_e, 0), (h_o, 64))):
                            nc.vector.tensor_tensor(
                                qkT[qc][po:po + 64, ic * 512:(ic + 1) * 512],
                                o_of[h][0:64, :],
                                bc_sb[po:po + 64, :],
                                MUL,
                            )

                # kick off weight prefetches for pair 0 before w_out so the
                # first qk matmuls aren't stuck behind 4MB of w_out traffic
                for f in range(KC + 1):
                    if f < KC:
                        emit_qk(f)
                    if f == 0:
                        nc.sync.dma_start(
                            w_out_sb[:],
                            w_out[:, :].rearrange("(ko p) f -> p ko f", p=128)
                            .bitcast(F32R),
                        )
                        nc.sync.dma_start(b_row[:], b_out[:].bitcast(F32R))
                    if f >= 1:
                        emit_attention_pair(f - 1)

                # ---------------- Phase C: out = cat.T @ w_out + b_out ------
                cat = qkT  # normalized attention output lives in the q tiles
                with tc.tile_pool(name="pc", bufs=2) as pc:
                    for half in range(2):
                        bb_ps = ps.tile([128, 512], FP32, tag="io", name="bb")
                        nc.tensor.matmul(
                            bb_ps[:],
                            ones[:],
                            b_row[:, half * 512:(half + 1) * 512],
                            start=True,
                            stop=True,
                        )
                        nc.vector.tensor_copy(
                            b_sb[:, half * 512:(half + 1) * 512], bb_ps[:]
                        )
                    for tc_i in range(8):
                        out_sb = pc.tile([128, D], FP32, tag="osb")
                        for mc in range(2):
                            c_ps = ps.tile([128, 512], FP32, tag="mm", name="cps")
                            for kc in range(KC):
                                nc.tensor.matmul(
                                    c_ps[:],
                                    cat[kc][:, tc_i * 128:(tc_i + 1) * 128],
                                    w_out_sb[:, kc, mc * 512:(mc + 1) * 512],
                                    start=(kc == 0),
                                    stop=(kc == KC - 1),
                                )
                            nc.vector.tensor_tensor(
                                out_sb[:, mc * 512:(mc + 1) * 512],
                                c_ps[:],
                                b_sb[:, mc * 512:(mc + 1) * 512],
                                ADD,
                            )
                        nc.sync.dma_start(
                            out[tc_i * 128:(tc_i + 1) * 128, :], out_sb[:]
                        )


_NC_CACHE = None


def _get_nc():
    global _NC_CACHE
    if _NC_CACHE is None:
        _NC_CACHE = build_attention_nc()
    return _NC_CACHE


def kernel(x, w_qkv, w_out, b_out, _trace=False, **_kw):
    x = np.ascontiguousarray(x, dtype=np.float32)
    w_qkv = np.ascontiguousarray(w_qkv, dtype=np.float32)
    w_out = np.ascontiguousarray(w_out, dtype=np.float32)
    b_row = np.ascontiguousarray(b_out, dtype=np.float32).reshape(1, D)

    nc = _get_nc()
    in_maps = [
        {"x": x[b], "w_qkv": w_qkv, "w_out": w_out, "b_out": b_row}
        for b in range(NC_)
    ]
    res = run_bass_kernel_spmd(nc, in_maps, core_ids=list(range(NC_)), trace=_trace)
    out = np.stack([res.results[b]["out"] for b in range(NC_)], axis=0)
    if _trace:
        return out, res
    return out


# revision 27
# speedup vs baseline: 1.0122x; 1.0122x over previous
"""Multi-head attention (b=8, n=1024, dim=1024, 16 heads x 64) on 8 TRN2 cores.

Sharding: data-parallel over batch (one batch element per core), SPMD NEFF.

Per-core dataflow (all matmuls f32r = full-speed PE with ~1e-4 matmul error):
  A) xT = PE-transpose(x) (f32r, batched 512-wide PSUM evacuations);
     v[t,f] = xT.T @ w_v written straight into a persistent SBUF tile in
     PV-stationary layout [keys-part, ko, h*65] with a ones column per head
     (no DRAM round-trip);
  B) qkT[f,t] = w_qk.T @ xT in head-pair order, software-pipelined with
     per-pair attention: S^T[j,i] = kT.T @ qT (K=64, two heads row-tiled
     concurrently); P^T = exp(scale*S^T) on ACT; [outT|l] = [v|1].T @ P^T
     (M=65, softmax denominator for free); normalize via reciprocal +
     K=33 ones-broadcast matmul; normalized output overwrites the dead q
     rows of qkT so cat == qkT.
  C) out = cat.T @ w_out + b_out; w_out is prefetched at phase-B start.

Scheduling: one PSUM pool (tags io/mm/s = 2+2+4 banks); phase-A SBUF pools
close before phase B so w_out + pipeline tiles reuse their space.
"""
import os
import sys

sys.path.insert(0, "/opt/trn_rl_repo")

import numpy as np

# diagnostic phase gating for HW attribution (default: full kernel)
# 1 = phase A only, 2 = A + qk projections, 3 = A + B, 4 = everything
_PHASES = int(os.environ.get("KPHASES", "4"))

import concourse.mybir as mybir
import concourse.tile as tile
from concourse import bacc
from concourse.bass_utils import run_bass_kernel_spmd
from concourse.masks import make_identity

FP32 = mybir.dt.float32
F32R = mybir.dt.float32r
AF = mybir.ActivationFunctionType
MUL = mybir.AluOpType.mult
ADD = mybir.AluOpType.add

N = 1024      # seq len
D = 1024      # model dim
H = 16        # heads
HD = 64       # head dim
SCALE = HD ** -0.5
NC_ = 8       # n cores = batch
KC = D // 128  # 8 contraction chunks


def build_attention_nc(reps=1):
    # reps>1 repeats the whole body in-NEFF -- used only by bench_hw.py to
    # measure per-exec time as a slope that cancels dispatch overhead.
    nc = bacc.Bacc("TRN2", target_bir_lowering=False, debug=False, num_devices=1)

    x = nc.dram_tensor("x", [N, D], FP32, kind="ExternalInput").ap()
    w_qkv = nc.dram_tensor("w_qkv", [D, 3 * D], FP32, kind="ExternalInput").ap()
    w_out = nc.dram_tensor("w_out", [D, D], FP32, kind="ExternalInput").ap()
    b_out = nc.dram_tensor("b_out", [1, D], FP32, kind="ExternalInput").ap()
    out = nc.dram_tensor("out", [N, D], FP32, kind="ExternalOutput").ap()

    with tile.TileContext(nc) as tc:
        for _rep in range(reps):
            _build_body(nc, tc, x, w_qkv, w_out, b_out, out)

    nc.compile()
    return nc


def _build_body(nc, tc, x, w_qkv, w_out, b_out, out):
        with (
            tc.tile_pool(name="persist", bufs=1) as pp,
            tc.tile_pool(name="ps", bufs=2, space="PSUM") as ps,
        ):
            ident = pp.tile([128, 128], FP32, tag="ident")
            make_identity(nc, ident[:])
            # f32r operands keep the broadcast matmuls at full PE speed
            # (fp32 matmuls run at 1/4 rate). memset can't write f32r, so
            # constants go through FP32 staging + converting copies.
            ones = pp.tile([1, 128], F32R, tag="ones")
            # head-pair selector for the paired 1/l broadcast: row 0 -> cols
            # 0-63 (even head), row 32 -> cols 64-127 (odd head)
            sel = pp.tile([33, 128], F32R, tag="sel")
            # reciprocal landing tiles: rows 0/32 are rewritten per use; rows
            # 1-31 must stay zero for the K=33 broadcast matmul
            rec_tiles = [
                pp.tile([33, 512], F32R, tag=f"rec{i}", name=f"rec{i}")
                for i in range(2)
            ]

            # xT[d % 128, kc, t]
            xT = pp.tile([128, KC, N], F32R, tag="xt", name="xT")
            qkT = [pp.tile([128, N], F32R, tag=f"qk{f}", name=f"qk{f}") for f in range(H)]
            # v in PV-stationary layout: [token%128, ko, h*65+e]; e==64 is 1.0
            v_sb = pp.tile([128, KC, H * 65], F32R, tag="vsb", name="v_sb")

            with tc.tile_pool(name="pi", bufs=1) as pi:
                sel_src = pi.tile([33, 128], FP32, tag="selsrc")
                nc.vector.memset(sel_src[:], 0.0)
                nc.vector.memset(sel_src[0:1, 0:64], 1.0)
                nc.vector.memset(sel_src[32:33, 64:128], 1.0)
                nc.vector.tensor_copy(sel[:], sel_src[:])
                zero_src = pi.tile([33, 512], FP32, tag="zerosrc")
                nc.vector.memset(zero_src[:], 0.0)
                for rt in rec_tiles:
                    nc.vector.tensor_copy(rt[:], zero_src[:])
                one_src = pi.tile([128, 128], FP32, tag="onesrc")
                nc.vector.memset(one_src[:], 1.0)
                nc.vector.tensor_copy(ones[:], one_src[0:1, :])
                nc.vector.tensor_copy(
                    v_sb[:].rearrange("p k (h e) -> p k h e", e=65)
                    [:, :, :, 64:65].rearrange("p k h e -> p k (h e)"),
                    one_src[:].rearrange("p (k h) -> p k h", k=KC),
                )

            # ---------------- Phase A: xT, v ----------------
            with tc.tile_pool(name="pa", bufs=1) as pa, \
                 tc.tile_pool(name="pa2", bufs=3) as pa2:
                # x chunk 0 first, then w_v's first half so v can start early
                x_tiles = [pa2.tile([128, D], FP32, tag="x", name="x0")]
                nc.sync.dma_start(x_tiles[0][:], x[0:128, :])
                wv = [pa.tile([128, KC, 512], F32R, tag=f"wv{fs}", name=f"wv{fs}") for fs in range(2)]
                nc.sync.dma_start(
                    wv[0][:],
                    w_qkv[:, 2 * D:2 * D + 512]
                    .rearrange("(ko p) f -> p ko f", p=128)
                    .bitcast(F32R),
                )
                for tc_i in range(1, 3):
                    x_sb = pa2.tile([128, D], FP32, tag="x", name=f"x{tc_i}")
                    nc.sync.dma_start(x_sb[:], x[tc_i * 128:(tc_i + 1) * 128, :])
                    x_tiles.append(x_sb)
                nc.sync.dma_start(
                    wv[1][:],
                    w_qkv[:, 2 * D + 512:2 * D + 1024]
                    .rearrange("(ko p) f -> p ko f", p=128)
                    .bitcast(F32R),
                )

                # warm the ACT exp table before phase B needs it
                warm = pa.tile([1, 2], FP32, tag="warm")
                nc.scalar.activation(warm[:], ident[0:1, 0:2], AF.Exp)

                for tc_i in range(8):
                    if tc_i >= 3:
                        x_sb = pa2.tile([128, D], FP32, tag="x", name=f"x{tc_i}")
                        nc.sync.dma_start(x_sb[:], x[tc_i * 128:(tc_i + 1) * 128, :])
                        x_tiles.append(x_sb)
                    x_sb = x_tiles[tc_i]
                    # transpose 4 d-chunks per PSUM tile, evacuate 512 wide
                    for half in range(2):
                        tr_ps = ps.tile([128, 512], FP32, tag="io", name="tr")
                        for q in range(4):
                            dc = half * 4 + q
                            nc.tensor.transpose(
                                tr_ps[:, q * 128:(q + 1) * 128],
                                x_sb[:, dc * 128:(dc + 1) * 128],
                                ident[:],
                            )
                        nc.vector.tensor_copy(
                            xT[:, half * 4:(half + 1) * 4, tc_i * 128:(tc_i + 1) * 128],
                            tr_ps[:].rearrange("p (a b) -> p a b", b=128),
                        )
                    for fs in range(2):
                        mm = ps.tile([128, 512], FP32, tag="mm")
                        for kc in range(KC):
                            nc.tensor.matmul(
                                mm[:],
                                xT[:, kc, tc_i * 128:(tc_i + 1) * 128],
                                wv[fs][:, kc, :],
                                start=(kc == 0),
                                stop=(kc == KC - 1),
                            )
                        nc.vector.tensor_copy(
                            v_sb[:, tc_i].rearrange("p (h e) -> p h e", e=65)
                            [:, fs * 8:(fs + 1) * 8, 0:64],
                            mm[:].rearrange("p (h e) -> p h e", e=64),
                        )

            # ---------------- Phase B/C pools ----------------
            with (
                tc.tile_pool(name="pwo", bufs=1) as pwo,
                tc.tile_pool(name="pbp", bufs=3) as pbp,
                tc.tile_pool(name="pb", bufs=2) as pb,
                tc.tile_pool(name="pb2", bufs=2) as pb2,
            ):
                # prefetch w_out + bias during phase B
                w_out_sb = pwo.tile([128, KC, D], F32R, tag="wo", name="w_out_sb")
                b_row = pwo.tile([1, D], F32R, tag="brow")
                b_sb = pwo.tile([128, D], FP32, tag="bsb")



                # Software pipeline: emit qk projection for head-pair f,
                # then the attention blocks for pair f-1. The exp stream on
                # ACT overlaps the qk matmuls on PE. Normalized attention
                # output is written back into the (dead) q rows of qkT, so
                # cat[c] IS qkT[c] (disjoint partition halves per head).
                def emit_qk(f):
                    for fc in (f, 8 + f):
                        wc = pb.tile([128, KC, 128], F32R, tag="wc")
                        nc.sync.dma_start(
                            wc[:],
                            w_qkv[:, fc * 128:(fc + 1) * 128]
                            .rearrange("(ko p) f -> p ko f", p=128)
                            .bitcast(F32R),
                        )
                        for ic in range(2):
                            mm = ps.tile([128, 512], FP32, tag="mm")
                            for kc in range(KC):
                                nc.tensor.matmul(
                                    mm[:],
                                    wc[:, kc, :],
                                    xT[:, kc, ic * 512:(ic + 1) * 512],
                                    start=(kc == 0),
                                    stop=(kc == KC - 1),
                                )
                            nc.vector.tensor_copy(qkT[fc][:, ic * 512:(ic + 1) * 512], mm[:])

                def emit_attention_pair(fp):
                    h_e, h_o = 2 * fp, 2 * fp + 1
                    qc = fp
                    qt, kt = qkT[qc], qkT[8 + qc]
                    for ic in range(2):
                        o_e = ps.tile([128, 512], FP32, tag="io", name="oe")
                        o_o = ps.tile([128, 512], FP32, tag="io", name="oo")
                        o_of = {h_e: o_e, h_o: o_o}
                        pts = []
                        # S matmuls for the two heads sit in disjoint PE row
                        # groups (partitions 0-63 / 64-127) -> HW-concurrent
                        for jc in range(KC):
                            s_ps = ps.tile([128, 2, 512], FP32, tag="s")
                            for hi, po in ((0, 0), (1, 64)):
                                nc.tensor.matmul(
                                    s_ps[:, hi, :],
                                    kt[po:po + 64, jc * 128:(jc + 1) * 128],
                                    qt[po:po + 64, ic * 512:(ic + 1) * 512],
                                    start=True,
                                    stop=True,
                                )
                            pt = pbp.tile([128, 2, 512], F32R, tag="pt")
                            nc.scalar.activation(
                                pt[:].rearrange("p a b -> p (a b)"),
                                s_ps[:].rearrange("p a b -> p (a b)"),
                                AF.Exp,
                                scale=SCALE,
                            )
                            pts.append(pt)
                            if jc >= 1:
                                for hi, h in ((0, h_e), (1, h_o)):
                                    nc.tensor.matmul(
                                        o_of[h][0:65, :],
                                        v_sb[:, jc - 1, h * 65:(h + 1) * 65],
                                        pts[jc - 1][:, hi, :],
                                        start=(jc - 1 == 0),
                                        stop=False,
                                    )
                        for hi, h in ((0, h_e), (1, h_o)):
                            nc.tensor.matmul(
                                o_of[h][0:65, :],
                                v_sb[:, KC - 1, h * 65:(h + 1) * 65],
                                pts[KC - 1][:, hi, :],
                                start=False,
                                stop=True,
                            )
                        rec2 = rec_tiles[ic]
                        # f32r out keeps the broadcast matmul at full PE rate;
                        # f32r is fp32-width so nothing is actually lost
                        with nc.allow_low_precision(reason="f32r == fp32 bits"):
                            nc.vector.reciprocal(rec2[0:1, :], o_e[64:65, :])
                            nc.vector.reciprocal(rec2[32:33, :], o_o[64:65, :])
                        b_ps = ps.tile([128, 512], FP32, tag="mm", name="bps")
                        nc.tensor.matmul(
                            b_ps[:], sel[:], rec2[:], start=True, stop=True
                        )
                        bc_sb = pb2.tile([128, 512], FP32, tag="bc")
                        nc.vector.tensor_copy(bc_sb[:], b_ps[:])
                        for hi, (h, po) in enumerate(((h_e, 0), (h_o, 64))):
                            nc.vector.tensor_tensor(
                                qkT[qc][po:po + 64, ic * 512:(ic + 1) * 512],
                                o_of[h][0:64, :],
                                bc_sb[po:po + 64, :],
                                MUL,
                            )

                # kick off weight prefetches for pair 0 before w_out so the
                # first qk matmuls aren't stuck behind 4MB of w_out traffic
                for f in range(KC + 1):
                    if f < KC and _PHASES >= 2:
                        emit_qk(f)
                    if f == 0 and _PHASES >= 4:
                        nc.sync.dma_start(
                            w_out_sb[:],
                            w_out[:, :].rearrange("(ko p) f -> p ko f", p=128)
                            .bitcast(F32R),
                        )
                        nc.sync.dma_start(b_row[:], b_out[:].bitcast(F32R))
                    if f >= 1 and _PHASES >= 3:
                        emit_attention_pair(f - 1)

                # ---------------- Phase C: out = cat.T @ w_out + b_out ------
                cat = qkT  # normalized attention output lives in the q tiles
                with tc.tile_pool(name="pc", bufs=2) as pc:
                    for half in range(2):
                        bb_ps = ps.tile([128, 512], FP32, tag="io", name="bb")
                        nc.tensor.matmul(
                            bb_ps[:],
                            ones[:],
                            b_row[:, half * 512:(half + 1) * 512],
                            start=True,
                            stop=True,
                        )
                        nc.vector.tensor_copy(
                            b_sb[:, half * 512:(half + 1) * 512], bb_ps[:]
                        )
                    for tc_i in range(8):
                        out_sb = pc.tile([128, D], FP32, tag="osb")
                        for mc in range(2):
                            c_ps = ps.tile([128, 512], FP32, tag="mm", name="cps")
                            for kc in range(KC):
                                nc.tensor.matmul(
                                    c_ps[:],
                                    cat[kc][:, tc_i * 128:(tc_i + 1) * 128],
                                    w_out_sb[:, kc, mc * 512:(mc + 1) * 512],
                                    start=(kc == 0),
                                    stop=(kc == KC - 1),
                                )
                            nc.vector.tensor_tensor(
                                out_sb[:, mc * 512:(mc + 1) * 512],
                                c_ps[:],
                                b_sb[:, mc * 512:(mc + 1) * 512],
                                ADD,
                            )
                        nc.sync.dma_start(
                            out[tc_i * 128:(tc_i + 1) * 128, :], out_sb[:]
                        )


_NC_CACHE = None


def _get_nc():
    global _NC_CACHE
    if _NC_CACHE is None:
        _NC_CACHE = build_attention_nc()
    return _NC_CACHE


def kernel(x, w_qkv, w_out, b_out, _trace=False, **_kw):
    x = np.ascontiguousarray(x, dtype=np.float32)
    w_qkv = np.ascontiguousarray(w_qkv, dtype=np.float32)
    w_out = np.ascontiguousarray(w_out, dtype=np.float32)
    b_row = np.ascontiguousarray(b_out, dtype=np.float32).reshape(1, D)

    nc = _get_nc()
    in_maps = [
        {"x": x[b], "w_qkv": w_qkv, "w_out": w_out, "b_out": b_row}
        for b in range(NC_)
    ]
    res = run_bass_kernel_spmd(nc, in_maps, core_ids=list(range(NC_)), trace=_trace)
    out = np.stack([res.results[b]["out"] for b in range(NC_)], axis=0)
    if _trace:
        return out, res
    return out
